# revision 1
# baseline (speedup 1.0000x reference)
"""BiViewMixHop GNN kernel for 8 Trainium2 NeuronCores (Bass/Tile).

Strategy:
  - Algebraic restructure: P(h)@W1 + P^2(h)@W2 = P(h@W1 + P(h@W2)); hom/het
    views fused into one 128-col tensor -> 2 gather passes per layer (6 total
    instead of 12 naive propagations).
  - Host prep (index manipulation only): relabel nodes into graph-aligned
    32-slot-padded "slots", shard whole graphs contiguously across 8 cores,
    sort each core's edges by dst slot, pad each 32-slot group's edge list to
    a multiple of 128 ("chunks"). Chunk counts per group are maxed across
    cores so ONE SPMD program serves all 8 cores (pad chunks carry zero
    masks and are no-ops numerically).
  - Gather: gpsimd indirect DMA, one [128,1]-offset DMA per 128-edge chunk,
    round-robined over 4 SWDGE queues (parallel descriptor streams).
  - Scatter/segment-sum: selection matrices S[e,d] = (dstloc==d)*mask built
    on DVE via broadcast tensor_tensor ops (3 ops per 16-chunk batch), then
    PE matmuls accumulate masked messages into per-group PSUM tiles.
  - Degree normalization: ones-vector matmuls in a prologue -> 1/deg,
    applied as per-partition ACT scale at PSUM evacuation.
  - Cross-core exchange of gather-source tensors via DRAM AllGather.
  - Readout: mean-pool via PE matmuls against one-hot graph selectors;
    max-pool via windowed reduce + per-graph mask+reduce; combined across
    cores with AllReduce; small MLP head + log_softmax replicated per core.
"""

import numpy as np

# ---------------------------------------------------------------- constants
F_IN = 128
H_HID = 64
N_LAYERS = 3
N_GRAPHS = 256
N_CLS = 10
NCORES = 8
P = 128
DGRP = 32           # dst-group width (selection-matrix columns)
SBATCH = 16         # chunks per S-build batch
W_G = 64            # per-core local-graph window
NQ = 4              # SWDGE queues
NGP = N_GRAPHS + 8  # pooled-buffer rows (graphs + dump row for pads)
DEBUG_DUMP = False
SKIP_GATHER = False
SKIP_AG = False


# ------------------------------------------------------------------ host prep
def _prep(edge_index, batch, hom_mask, het_mask):
    """Index-only preprocessing. Returns structure dict + per-core arrays."""
    N = batch.shape[0]
    src, dst = np.asarray(edge_index[0]), np.asarray(edge_index[1])
    batch = np.asarray(batch)
    counts = np.bincount(batch, minlength=N_GRAPHS)

    pad_sizes = ((counts + DGRP - 1) // DGRP) * DGRP
    total_slots = int(pad_sizes.sum())
    target = max(total_slots / NCORES, 1.0)

    csum = np.concatenate([[0], np.cumsum(pad_sizes)])
    gcore = np.minimum((csum[:-1] + pad_sizes / 2) / target, NCORES - 1).astype(np.int64)
    gcore = np.maximum.accumulate(gcore)

    core_slots = np.zeros(NCORES, np.int64)
    graph_base = np.zeros(N_GRAPHS, np.int64)
    for g in range(N_GRAPHS):
        c = gcore[g]
        graph_base[g] = core_slots[c]
        core_slots[c] += pad_sizes[g]
    S_core = int(((core_slots.max() + P - 1) // P) * P)
    TILES = S_core // P
    GROUPS = S_core // DGRP
    TOT = NCORES * S_core

    node_starts = np.concatenate([[0], np.cumsum(counts)])
    node_slot = np.empty(N, np.int64)
    for g in range(N_GRAPHS):
        a, b = node_starts[g], node_starts[g + 1]
        if b > a:
            base = gcore[g] * S_core + graph_base[g]
            node_slot[a:b] = base + np.arange(b - a)

    src_slot = node_slot[src]
    dst_slot = node_slot[dst]
    owner = dst_slot // S_core

    core_graphs = [np.where((gcore == c) & (counts > 0))[0] for c in range(NCORES)]
    n_local = max(len(cg) for cg in core_graphs)
    assert n_local <= W_G, f"{n_local} local graphs > W_G={W_G}"

    cpg = np.ones(GROUPS, np.int64)
    per_core_edges = []
    hom_mask = np.asarray(hom_mask)
    het_mask = np.asarray(het_mask)
    for c in range(NCORES):
        m = owner == c
        es, ed = src_slot[m], dst_slot[m]
        eh, et = hom_mask[m], het_mask[m]
        order = np.argsort(ed, kind="stable")
        es, ed, eh, et = es[order], ed[order], eh[order], et[order]
        loc = ed - c * S_core
        grp = loc // DGRP
        gstart = np.searchsorted(grp, np.arange(GROUPS))
        gend = np.searchsorted(grp, np.arange(GROUPS) + 1)
        per_core_edges.append((es, loc, eh, et, gstart, gend))
        cnt = gend - gstart
        cpg = np.maximum(cpg, (cnt + P - 1) // P)
    C_CH = int(cpg.sum())
    cog = np.concatenate([[0], np.cumsum(cpg)])

    st = {"S_core": S_core, "TILES": TILES, "GROUPS": GROUPS, "TOT": TOT,
          "C_CH": C_CH, "cpg": cpg, "cog": cog, "node_slot": node_slot,
          "counts": counts}

    per_core = []
    for c in range(NCORES):
        es, loc, eh, et, gstart, gend = per_core_edges[c]
        srcA = np.zeros((P, C_CH), np.int32)
        dlA = np.zeros((P, C_CH), np.float32)
        mhA = np.zeros((P, C_CH), np.float32)
        mtA = np.zeros((P, C_CH), np.float32)
        for g in range(GROUPS):
            a, b = gstart[g], gend[g]
            c0 = cog[g]
            for j in range(cpg[g]):
                lo = a + j * P
                hi = min(a + (j + 1) * P, b)
                if hi <= lo:
                    break
                k = hi - lo
                srcA[:k, c0 + j] = es[lo:hi]
                dlA[:k, c0 + j] = (loc[lo:hi] - g * DGRP).astype(np.float32)
                mhA[:k, c0 + j] = eh[lo:hi]
                mtA[:k, c0 + j] = et[lo:hi]

        batchloc = np.full(S_core, -1.0, np.float32)
        maskrow = np.zeros((W_G, GROUPS), np.float32)
        gidx = np.full((W_G, 1), N_GRAPHS, np.int32)  # pads -> dump row
        for li, g in enumerate(core_graphs[c]):
            base = graph_base[g]
            batchloc[base:base + counts[g]] = li
            g0, g1 = base // DGRP, (base + pad_sizes[g]) // DGRP
            maskrow[li, g0:g1] = 1.0
            gidx[li, 0] = g
        per_core.append({
            "src": srcA, "dl": dlA, "mh": mhA, "mt": mtA,
            "batchloc": batchloc.reshape(TILES, P).T.copy(),
            "maskrow": maskrow, "gidx": gidx})
    return st, per_core


# ------------------------------------------------------------- device builder
def _build(st):
    import concourse.bass as bass
    import concourse.bacc as bacc
    import concourse.mybir as mybir
    import concourse.tile as tile
    from concourse.masks import make_identity

    S_core, TILES, GROUPS = st["S_core"], st["TILES"], st["GROUPS"]
    TOT, C_CH, cog = st["TOT"], st["C_CH"], st["cog"]
    NB = (C_CH + SBATCH - 1) // SBATCH
    f32 = mybir.dt.float32
    F2 = 2 * H_HID  # 128

    nc = bacc.Bacc("TRN2", target_bir_lowering=False, debug=False,
                   num_devices=NCORES, num_swdge_queues=NQ)

    def din(name, shape, dtype=f32):
        return nc.dram_tensor(name, shape, dtype, kind="ExternalInput").ap()

    x_own = din("x_own", [S_core, F_IN])
    srcA = din("srcA", [P, C_CH], mybir.dt.int32)
    dlA = din("dlA", [P, C_CH])
    mhA = din("mhA", [P, C_CH])
    mtA = din("mtA", [P, C_CH])
    batchloc = din("batchloc", [P, TILES])
    maskrow = din("maskrow", [W_G, GROUPS])
    gidx = din("gidx", [W_G, 1], mybir.dt.int32)
    rcount_g = din("rcount_g", [NGP, 1])
    w2cat = din("w2cat", [N_LAYERS, F_IN, F2])
    w1cat = din("w1cat", [N_LAYERS, F_IN, F2])
    w0cat = din("w0cat", [N_LAYERS, F_IN, F2])
    bcat = din("bcat", [N_LAYERS, 1, F2])
    lin1w = din("lin1w", [4 * H_HID, 2 * H_HID])
    lin1b = din("lin1b", [2 * H_HID, 1])
    lin2w = din("lin2w", [2 * H_HID, H_HID])
    lin2b = din("lin2b", [H_HID, 1])
    lin3w = din("lin3w", [H_HID, N_CLS])
    lin3b = din("lin3b", [N_CLS, 1])

    out = nc.dram_tensor("out", [N_GRAPHS, N_CLS], f32, kind="ExternalOutput").ap()
    if DEBUG_DUMP:
        dbg_c = nc.dram_tensor("dbg_c", [S_core, F_IN], f32, kind="ExternalOutput").ap()
        dbg_u = nc.dram_tensor("dbg_u", [S_core, F_IN], f32, kind="ExternalOutput").ap()
        dbg_h = nc.dram_tensor("dbg_h", [S_core, F_IN], f32, kind="ExternalOutput").ap()
        dbg_rd = nc.dram_tensor("dbg_rd", [P, 64], f32, kind="ExternalOutput").ap()

    c_own = nc.dram_tensor("c_own", [S_core, F_IN], f32).ap()
    u_own = nc.dram_tensor("u_own", [S_core, F_IN], f32).ap()
    c_full = nc.dram_tensor("c_full", [TOT, F_IN], f32, addr_space="Shared").ap()
    u_full = nc.dram_tensor("u_full", [TOT, F_IN], f32, addr_space="Shared").ap()
    maxbuf = nc.dram_tensor("maxbuf", [NGP, F_IN], f32).ap()
    sumbuf = nc.dram_tensor("sumbuf", [NGP, F_IN], f32).ap()
    maxbuf_o = nc.dram_tensor("maxbuf_o", [NGP, F_IN], f32, addr_space="Shared").ap()
    sumbuf_o = nc.dram_tensor("sumbuf_o", [NGP, F_IN], f32, addr_space="Shared").ap()

    def indirect_gather(out_ap, table_ap, off_ap, qi):
        eng = nc.gpsimd
        out_l = eng.lower_ap_dma(out_ap, for_indirect_dma=True)
        in_l = eng.lower_ap_dma(table_ap, for_indirect_dma=True)
        off_l = eng.lower_ap_dma(off_ap)
        in_l.append(off_l[0])
        coef = int(np.prod(table_ap.shape[1:]))
        dyn = mybir.DynamicAccessPatternInfo(
            c=0, actual_ap=out_ap.ap, indirect_dim_max_index=table_ap.shape[0],
            offset_expr=[mybir.DynamicAccessPatternOffsetExpr(
                coef=coef,
                aff_expr=mybir.DynamicAccessPatternOffsetExprAffExpr(
                    kind="IndirectArgId", arg_id=1))])
        in_l[0].dynamic_ap_info = dyn
        return eng.add_instruction(mybir.InstDMACopy(
            name=nc.get_next_instruction_name(),
            queue="qPoolDynamic" + (str(qi) if qi else ""), mode="Copy",
            ins=in_l, outs=out_l, oob_is_err=True,
            cce_op=mybir.AluOpType.bypass))

    with tile.TileContext(nc) as tc:
        with tc.tile_pool(name="const", bufs=1) as cpool, \
             tc.tile_pool(name="sb", bufs=2) as spool, \
             tc.tile_pool(name="gt", bufs=8) as gpool, \
             tc.tile_pool(name="stg", bufs=3) as stgpool, \
             tc.tile_pool(name="sm", bufs=3) as smpool, \
             tc.tile_pool(name="psA", bufs=2, space="PSUM") as psA, \
             tc.tile_pool(name="psB", bufs=2, space="PSUM") as psB, \
             tc.tile_pool(name="psC", bufs=2, space="PSUM") as psC, \
             tc.tile_pool(name="psP", bufs=1, space="PSUM") as psP:

            # ---------------- resident tiles
            ident = cpool.tile([P, P], f32)
            make_identity(nc, ident[:])
            src_t = cpool.tile([P, C_CH], mybir.dt.int32)
            nc.sync.dma_start(src_t[:], srcA[:])
            dl_t = cpool.tile([P, C_CH], f32)
            nc.sync.dma_start(dl_t[:], dlA[:])
            mh_t = cpool.tile([P, C_CH], f32)
            nc.sync.dma_start(mh_t[:], mhA[:])
            mt_t = cpool.tile([P, C_CH], f32)
            nc.sync.dma_start(mt_t[:], mtA[:])
            iota32_i = cpool.tile([P, SBATCH * DGRP], mybir.dt.int32)
            nc.gpsimd.iota(iota32_i[:].rearrange("p (k d) -> p k d", d=DGRP),
                           pattern=[[0, SBATCH], [1, DGRP]], base=0,
                           channel_multiplier=0)
            iota32 = cpool.tile([P, SBATCH * DGRP], f32)
            nc.vector.tensor_copy(iota32[:], iota32_i[:])
            iotaWG_i = cpool.tile([P, W_G], mybir.dt.int32)
            nc.gpsimd.iota(iotaWG_i[:], pattern=[[1, W_G]], base=0,
                           channel_multiplier=0)
            iotaWG = cpool.tile([P, W_G], f32)
            nc.vector.tensor_copy(iotaWG[:], iotaWG_i[:])
            ones_col = cpool.tile([1, P], f32)
            nc.vector.memset(ones_col[:], 1.0)
            onesP = cpool.tile([P, 2], f32)
            nc.vector.memset(onesP[:], 1.0)
            zero_t = cpool.tile([P, P], f32)
            nc.vector.memset(zero_t[:], 0.0)
            hT = cpool.tile([P, S_core], f32)
            rdeg = cpool.tile([P, 2 * TILES], f32)
            bl_t = cpool.tile([P, TILES], f32)
            nc.sync.dma_start(bl_t[:], batchloc[:])
            mrow_t = cpool.tile([W_G, GROUPS], f32)
            nc.sync.dma_start(mrow_t[:], maskrow[:])
            gidx_t = cpool.tile([W_G, 1], mybir.dt.int32)
            nc.sync.dma_start(gidx_t[:], gidx[:])
            rcg_t = cpool.tile([P, 2], f32)
            nc.sync.dma_start(
                rcg_t[:], rcount_g[:2 * P, :].rearrange("(a b) o -> b (a o)", a=2))
            wAll = cpool.tile([P, 9 * F2], f32)
            for l in range(N_LAYERS):
                nc.sync.dma_start(wAll[:, (3 * l + 0) * F2:(3 * l + 1) * F2], w2cat[l])
                nc.sync.dma_start(wAll[:, (3 * l + 1) * F2:(3 * l + 2) * F2], w1cat[l])
                nc.sync.dma_start(wAll[:, (3 * l + 2) * F2:(3 * l + 3) * F2], w0cat[l])
            bAll = cpool.tile([1, N_LAYERS * F2], f32)
            for l in range(N_LAYERS):
                nc.sync.dma_start(bAll[:, l * F2:(l + 1) * F2], bcat[l])
            l1w = cpool.tile([P, 2 * F2], f32)  # two K-halves side by side
            nc.sync.dma_start(l1w[:, 0:F2], lin1w[0:P, :])
            nc.sync.dma_start(l1w[:, F2:2 * F2], lin1w[P:2 * P, :])
            l2w = cpool.tile([2 * H_HID, H_HID], f32)
            nc.sync.dma_start(l2w[:], lin2w[:])
            l2b = cpool.tile([H_HID, 1], f32)
            nc.sync.dma_start(l2b[:], lin2b[:])
            l1b = cpool.tile([2 * H_HID, 1], f32)
            nc.sync.dma_start(l1b[:], lin1b[:])
            l3w = cpool.tile([H_HID, N_CLS], f32)
            nc.sync.dma_start(l3w[:], lin3w[:])
            l3b = cpool.tile([N_CLS, 1], f32)
            nc.sync.dma_start(l3b[:], lin3b[:])
            spool_t = cpool.tile([P, TILES * W_G], f32)
            gmax12 = cpool.tile([P, W_G], f32)
            nc.vector.memset(gmax12[:], 0.0)
            gt0 = None
            if SKIP_GATHER:
                gt0 = cpool.tile([P, F_IN], f32)
                nc.vector.memset(gt0[:], 0.25)

            def wslice(l, which):  # 0=w2, 1=w1, 2=w0
                o = (3 * l + which) * F2
                return wAll[:, o:o + F2]

            def build_S(b):
                c0 = b * SBATCH
                nch = min(SBATCH, C_CH - c0)
                eq = spool.tile([P, SBATCH * DGRP], f32, tag="eq")
                sh = spool.tile([P, SBATCH * DGRP], f32, tag="sh")
                stt = spool.tile([P, SBATCH * DGRP], f32, tag="st")
                r3 = lambda ap: ap.rearrange("p (k d) -> p k d", d=DGRP)[:, :nch, :]
                nc.vector.tensor_tensor(
                    out=r3(eq[:]),
                    in0=dl_t[:, c0:c0 + nch, None].to_broadcast([P, nch, DGRP]),
                    in1=r3(iota32[:]), op=mybir.AluOpType.is_equal)
                nc.vector.tensor_tensor(
                    out=r3(sh[:]), in0=r3(eq[:]),
                    in1=mh_t[:, c0:c0 + nch, None].to_broadcast([P, nch, DGRP]),
                    op=mybir.AluOpType.mult)
                nc.vector.tensor_tensor(
                    out=r3(stt[:]), in0=r3(eq[:]),
                    in1=mt_t[:, c0:c0 + nch, None].to_broadcast([P, nch, DGRP]),
                    op=mybir.AluOpType.mult)
                return sh, stt

            # ---------------- degree prologue
            # NOTE: PSUM accumulation groups must not interleave within one
            # bank -> run all hom-chunk matmuls to completion, then het.
            S_cache = {}

            def get_S(b):
                if b not in S_cache:
                    S_cache[b] = build_S(b)
                    for k in [k for k in S_cache if k < b - 1]:
                        del S_cache[k]
                return S_cache[b]

            for g in range(GROUPS):
                pd_cur = psA.tile([DGRP, 2], f32, tag="t")
                chunks = list(range(cog[g], cog[g + 1]))
                for j in chunks:
                    sh, _ = get_S(j // SBATCH)
                    jj = j % SBATCH
                    nc.tensor.matmul(pd_cur[:, 0:1],
                                     lhsT=sh[:, jj * DGRP:(jj + 1) * DGRP],
                                     rhs=onesP[:, 0:1], start=j == chunks[0],
                                     stop=j == chunks[-1])
                for j in chunks:
                    _, stt = get_S(j // SBATCH)
                    jj = j % SBATCH
                    nc.tensor.matmul(pd_cur[:, 1:2],
                                     lhsT=stt[:, jj * DGRP:(jj + 1) * DGRP],
                                     rhs=onesP[:, 1:2], start=j == chunks[0],
                                     stop=j == chunks[-1])
                r0 = (g % 4) * DGRP
                t0 = g // 4
                nc.scalar.copy(rdeg[r0:r0 + DGRP, 2 * t0:2 * t0 + 2], pd_cur[:])
            S_cache.clear()
            nc.vector.tensor_scalar_max(rdeg[:], rdeg[:], 1.0)
            nc.vector.reciprocal(rdeg[:], rdeg[:])
            if DEBUG_DUMP:
                nc.sync.dma_start(dbg_rd[:, 0:2 * TILES], rdeg[:])

            # ---------------- x -> hT (feature-major)
            for t in range(TILES):
                xs = stgpool.tile([P, P], f32, tag="xs")
                nc.sync.dma_start(xs[:], x_own[t * P:(t + 1) * P, :])
                pt = psC.tile([P, 2 * P], f32, tag="c")
                nc.tensor.transpose(pt[:, 0:P], xs[:], ident[:])
                nc.scalar.copy(hT[:, t * P:(t + 1) * P], pt[:, 0:P])

            # ---------------- S_pool (one-hot local-graph selectors)
            for t in range(TILES):
                nc.vector.tensor_tensor(
                    out=spool_t[:, t * W_G:(t + 1) * W_G],
                    in0=bl_t[:, t:t + 1].to_broadcast([P, W_G]),
                    in1=iotaWG[:], op=mybir.AluOpType.is_equal)

            # ---------------- zero-fill pooled buffers
            for buf in (maxbuf, sumbuf):
                r = 0
                while r < NGP:
                    k = min(P, NGP - r)
                    nc.sync.dma_start(buf[r:r + k, :], zero_t[:k, :])
                    r += k

            pool_ps = psP.tile([W_G, F_IN], f32)

            def produce(l, dest):
                for t in range(TILES):
                    pc = psC.tile([P, 2 * P], f32, tag="c")
                    nc.tensor.matmul(pc[:, 0:F2], lhsT=hT[:, t * P:(t + 1) * P],
                                     rhs=wslice(l, 0), start=True, stop=True)
                    cs = stgpool.tile([P, F2], f32, tag="cs")
                    nc.scalar.copy(cs[:], pc[:, 0:F2])
                    nc.sync.dma_start(dest[t * P:(t + 1) * P, :], cs[:])

            def prop_pass(l, table, mode, do_pool):
                qctr = 0
                S_cache2 = {}

                def get_S2(b):
                    if b not in S_cache2:
                        S_cache2[b] = build_S(b)
                        for k in [k for k in S_cache2 if k < b - 1]:
                            del S_cache2[k]
                    return S_cache2[b]

                for t in range(TILES):
                    stg = stgpool.tile([P, P], f32, tag="hstg")
                    for gi in range(4):
                        g = t * 4 + gi
                        ps_t = psA.tile([DGRP, F_IN], f32, tag="t")
                        chunks = list(range(cog[g], cog[g + 1]))
                        gts = []
                        for j in chunks:
                            sh, _ = get_S2(j // SBATCH)
                            jj = j % SBATCH
                            if SKIP_GATHER:
                                gt = gt0
                            else:
                                gt = gpool.tile([P, F_IN], f32, tag="g")
                                indirect_gather(gt[:], table, src_t[:, j:j + 1],
                                                qctr % NQ)
                            qctr += 1
                            gts.append(gt)
                            nc.tensor.matmul(
                                ps_t[:, 0:H_HID],
                                lhsT=sh[:, jj * DGRP:(jj + 1) * DGRP],
                                rhs=gt[:, 0:H_HID], start=j == chunks[0],
                                stop=j == chunks[-1])
                        for gt, j in zip(gts, chunks):
                            _, stt = get_S2(j // SBATCH)
                            jj = j % SBATCH
                            nc.tensor.matmul(
                                ps_t[:, H_HID:F_IN],
                                lhsT=stt[:, jj * DGRP:(jj + 1) * DGRP],
                                rhs=gt[:, H_HID:F_IN], start=j == chunks[0],
                                stop=j == chunks[-1])
                        ps_a = psB.tile([DGRP, F_IN], f32, tag="a")
                        nc.tensor.matmul(
                            ps_a[:], lhsT=hT[:, g * DGRP:(g + 1) * DGRP],
                            rhs=wslice(l, 1 if mode == "A" else 2),
                            start=True, stop=(mode == "A"))
                        if mode == "B":
                            nc.tensor.matmul(ps_a[:], lhsT=ones_col[:, 0:DGRP],
                                             rhs=bAll[:, l * F2:(l + 1) * F2],
                                             start=False, stop=True)
                        r0 = (g % 4) * DGRP
                        t0 = g // 4
                        tp = smpool.tile([DGRP, F_IN], f32, tag="tp")
                        nc.scalar.mul(tp[:, 0:H_HID], ps_t[:, 0:H_HID],
                                      rdeg[r0:r0 + DGRP, 2 * t0:2 * t0 + 1])
                        nc.scalar.mul(tp[:, H_HID:F_IN], ps_t[:, H_HID:F_IN],
                                      rdeg[r0:r0 + DGRP, 2 * t0 + 1:2 * t0 + 2])
                        dst_rows = stg[gi * DGRP:(gi + 1) * DGRP, :]
                        nc.vector.tensor_tensor(out=dst_rows, in0=tp[:],
                                                in1=ps_a[:],
                                                op=mybir.AluOpType.add)
                        if mode == "B":
                            nc.vector.tensor_scalar_max(dst_rows, dst_rows, 0.0)
                    if mode == "A":
                        nc.sync.dma_start(u_own[t * P:(t + 1) * P, :], stg[:])
                        if DEBUG_DUMP and l == 0:
                            nc.sync.dma_start(dbg_u[t * P:(t + 1) * P, :], stg[:])
                    else:
                        if DEBUG_DUMP and l == 0:
                            nc.sync.dma_start(dbg_h[t * P:(t + 1) * P, :], stg[:])
                        ptr = psC.tile([P, 2 * P], f32, tag="c")
                        nc.tensor.transpose(ptr[:, 0:P], stg[:], ident[:])
                        nc.scalar.copy(hT[:, t * P:(t + 1) * P], ptr[:, 0:P])
                        if do_pool:
                            nc.tensor.matmul(
                                pool_ps[:],
                                lhsT=spool_t[:, t * W_G:(t + 1) * W_G],
                                rhs=stg[:],
                                start=(l == 1 and t == 0),
                                stop=(l == N_LAYERS - 1 and t == TILES - 1))

            def max_pool_layer():
                m1 = smpool.tile([P, GROUPS], f32, tag="m1")
                nc.vector.tensor_reduce(
                    out=m1[:], in_=hT[:].rearrange("p (g d) -> p g d", d=DGRP),
                    axis=mybir.AxisListType.X, op=mybir.AluOpType.max)
                for li in range(W_G):
                    mrow_row = smpool.tile([1, GROUPS], f32, tag="mrow")
                    nc.sync.dma_start(mrow_row[:], maskrow[li:li + 1, :])
                    mb = psC.tile([P, 2 * P], f32, tag="c")
                    nc.tensor.matmul(mb[:, 0:GROUPS], lhsT=ones_col[:],
                                     rhs=mrow_row[:], start=True,
                                     stop=True)
                    msel = smpool.tile([P, GROUPS], f32, tag="msel")
                    nc.vector.tensor_tensor(out=msel[:], in0=m1[:],
                                            in1=mb[:, 0:GROUPS],
                                            op=mybir.AluOpType.mult)
                    gm = smpool.tile([P, 1], f32, tag="gm")
                    nc.vector.tensor_reduce(out=gm[:], in_=msel[:],
                                            axis=mybir.AxisListType.X,
                                            op=mybir.AluOpType.max)
                    nc.vector.tensor_tensor(out=gmax12[:, li:li + 1],
                                            in0=gmax12[:, li:li + 1], in1=gm[:],
                                            op=mybir.AluOpType.add)

            # ================ main layer loop
            for l in range(N_LAYERS):
                produce(l, c_own)
                if DEBUG_DUMP and l == 0:
                    for t in range(TILES):
                        cs2 = stgpool.tile([P, F_IN], f32, tag="cs")
                        nc.sync.dma_start(cs2[:], c_own[t * P:(t + 1) * P, :])
                        nc.sync.dma_start(dbg_c[t * P:(t + 1) * P, :], cs2[:])
                if not SKIP_AG:
                    nc.gpsimd.collective_compute(
                        "AllGather", mybir.AluOpType.bypass,
                        ins=[c_own[:]], outs=[c_full[:]],
                        replica_groups=[list(range(NCORES))])
                prop_pass(l, c_full[:], "A", False)
                if not SKIP_AG:
                    nc.gpsimd.collective_compute(
                        "AllGather", mybir.AluOpType.bypass,
                        ins=[u_own[:]], outs=[u_full[:]],
                        replica_groups=[list(range(NCORES))])
                prop_pass(l, u_full[:], "B", l >= 1)
                if l >= 1:
                    max_pool_layer()

            # ================ pooled outputs -> DRAM -> AllReduce
            sums = smpool.tile([W_G, F_IN], f32, tag="sums")
            nc.scalar.copy(sums[:], pool_ps[:])
            nc.gpsimd.indirect_dma_start(
                out=sumbuf[:],
                out_offset=bass.IndirectOffsetOnAxis(ap=gidx_t[:, 0:1], axis=0),
                in_=sums[:], in_offset=None)
            pmx = psC.tile([P, 2 * P], f32, tag="c")
            nc.tensor.transpose(pmx[0:W_G, 0:P], gmax12[:], ident[:])
            mxs = smpool.tile([W_G, P], f32, tag="mxs")
            nc.scalar.copy(mxs[:], pmx[0:W_G, 0:P])
            nc.gpsimd.indirect_dma_start(
                out=maxbuf[:],
                out_offset=bass.IndirectOffsetOnAxis(ap=gidx_t[:, 0:1], axis=0),
                in_=mxs[:], in_offset=None)
            nc.gpsimd.collective_compute(
                "AllReduce", mybir.AluOpType.max,
                ins=[maxbuf[:]], outs=[maxbuf_o[:]],
                replica_groups=[list(range(NCORES))])
            nc.gpsimd.collective_compute(
                "AllReduce", mybir.AluOpType.add,
                ins=[sumbuf[:]], outs=[sumbuf_o[:]],
                replica_groups=[list(range(NCORES))])

            # ================ head (replicated)
            rT = smpool.tile([P, 4 * P], f32, tag="rT")  # [feat128, max256|mean256]
            for half in range(2):
                mx = smpool.tile([P, F_IN], f32, tag="mx")
                nc.sync.dma_start(mx[:], maxbuf_o[half * P:(half + 1) * P, :])
                sm = smpool.tile([P, F_IN], f32, tag="smh")
                nc.sync.dma_start(sm[:], sumbuf_o[half * P:(half + 1) * P, :])
                nc.vector.tensor_scalar(out=sm[:], in0=sm[:],
                                        scalar1=rcg_t[:, half:half + 1],
                                        scalar2=None, op0=mybir.AluOpType.mult)
                pmxT = psC.tile([P, 2 * P], f32, tag="c")
                nc.tensor.transpose(pmxT[:, 0:P], mx[:], ident[:])
                nc.scalar.copy(rT[:, half * P:(half + 1) * P], pmxT[:, 0:P])
                psmT = psC.tile([P, 2 * P], f32, tag="c")
                nc.tensor.transpose(psmT[:, 0:P], sm[:], ident[:])
                nc.scalar.copy(rT[:, 2 * P + half * P:2 * P + (half + 1) * P],
                               psmT[:, 0:P])

            z1p = psC.tile([P, 2 * P], f32, tag="c")
            nc.tensor.matmul(z1p[:F2, 0:2 * P], lhsT=l1w[:, 0:F2],
                             rhs=rT[:, 0:2 * P], start=True, stop=False)
            nc.tensor.matmul(z1p[:F2, 0:2 * P], lhsT=l1w[:, F2:2 * F2],
                             rhs=rT[:, 2 * P:4 * P], start=False, stop=True)
            z1 = smpool.tile([F2, 2 * P], f32, tag="z1")
            nc.scalar.activation(z1[:], z1p[:F2, 0:2 * P],
                                 mybir.ActivationFunctionType.Relu,
                                 bias=l1b[:, 0:1], scale=1.0)
            z2p = psC.tile([P, 2 * P], f32, tag="c")
            nc.tensor.matmul(z2p[:H_HID, 0:2 * P], lhsT=l2w[:], rhs=z1[:],
                             start=True, stop=True)
            z2 = smpool.tile([H_HID, 2 * P], f32, tag="z2")
            nc.scalar.activation(z2[:], z2p[:H_HID, 0:2 * P],
                                 mybir.ActivationFunctionType.Relu,
                                 bias=l2b[:, 0:1], scale=1.0)
            z3p = psC.tile([P, 2 * P], f32, tag="c")
            nc.tensor.matmul(z3p[:N_CLS, 0:2 * P], lhsT=l3w[:], rhs=z2[:],
                             start=True, stop=True)
            z3 = smpool.tile([N_CLS, 2 * P], f32, tag="z3")
            nc.scalar.activation(z3[:], z3p[:N_CLS, 0:2 * P],
                                 mybir.ActivationFunctionType.Identity,
                                 bias=l3b[:, 0:1], scale=1.0)
            for half in range(2):
                lg = psC.tile([P, 2 * P], f32, tag="c")
                nc.tensor.transpose(lg[:, 0:N_CLS],
                                    z3[:, half * P:(half + 1) * P],
                                    ident[0:N_CLS, 0:N_CLS])
                lgs = smpool.tile([P, N_CLS], f32, tag="lgs")
                nc.vector.tensor_copy(lgs[:], lg[:, 0:N_CLS])
                rmax = smpool.tile([P, 1], f32, tag="rmax")
                nc.vector.tensor_reduce(out=rmax[:], in_=lgs[:],
                                        axis=mybir.AxisListType.X,
                                        op=mybir.AluOpType.max)
                xm = smpool.tile([P, N_CLS], f32, tag="xm")
                nc.vector.tensor_scalar(out=xm[:], in0=lgs[:],
                                        scalar1=rmax[:, 0:1], scalar2=None,
                                        op0=mybir.AluOpType.subtract)
                ex = smpool.tile([P, N_CLS], f32, tag="ex")
                nc.scalar.activation(ex[:], xm[:],
                                     mybir.ActivationFunctionType.Exp)
                sume = smpool.tile([P, 1], f32, tag="sume")
                nc.vector.tensor_reduce(out=sume[:], in_=ex[:],
                                        axis=mybir.AxisListType.X,
                                        op=mybir.AluOpType.add)
                lse = smpool.tile([P, 1], f32, tag="lse")
                nc.scalar.activation(lse[:], sume[:],
                                     mybir.ActivationFunctionType.Ln)
                res = smpool.tile([P, N_CLS], f32, tag="res")
                nc.vector.tensor_scalar(out=res[:], in0=xm[:],
                                        scalar1=lse[:, 0:1], scalar2=None,
                                        op0=mybir.AluOpType.subtract)
                nc.sync.dma_start(out[half * P:(half + 1) * P, :], res[:])

    nc.compile()
    return nc


# ------------------------------------------------------------------ runner
def _make_runner(nc, n_cores):
    import jax
    import concourse.mybir as mybir
    from jax.experimental.shard_map import shard_map
    from jax.sharding import Mesh, NamedSharding, PartitionSpec
    from concourse.bass2jax import (_bass_exec_p, install_neuronx_cc_hook,
                                    partition_id_tensor)

    install_neuronx_cc_hook()
    partition_name = nc.partition_id_tensor.name if nc.partition_id_tensor else None
    in_names, out_names, out_avals = [], [], []
    for alloc in nc.m.functions[0].allocations:
        if not isinstance(alloc, mybir.MemoryLocationSet):
            continue
        name = alloc.memorylocations[0].name
        if alloc.kind == "ExternalInput":
            if name != partition_name:
                in_names.append(name)
        elif alloc.kind == "ExternalOutput":
            out_names.append(name)
            out_avals.append(jax.core.ShapedArray(
                tuple(alloc.tensor_shape), mybir.dt.np(alloc.dtype)))
    n_params = len(in_names)
    all_in = list(in_names) + list(out_names)
    if partition_name is not None:
        all_in.append(partition_name)

    def _body(*args):
        operands = list(args)
        if partition_name is not None:
            operands.append(partition_id_tensor())
        return tuple(_bass_exec_p.bind(
            *operands, out_avals=tuple(out_avals), in_names=tuple(all_in),
            out_names=tuple(out_names), lowering_input_output_aliases=(),
            sim_require_finite=False, sim_require_nnan=False, nc=nc))

    devices = jax.devices()[:n_cores]
    mesh = Mesh(np.asarray(devices), ("core",))
    nin = n_params + len(out_names)
    sharded = jax.jit(shard_map(
        _body, mesh=mesh, in_specs=(PartitionSpec("core"),) * nin,
        out_specs=(PartitionSpec("core"),) * len(out_names), check_rep=False),
        keep_unused=True)
    sharding = NamedSharding(mesh, PartitionSpec("core"))

    def stage(in_maps):
        import jax as _jax
        concat_in = [np.concatenate([np.asarray(in_maps[c][nm])
                                     for c in range(n_cores)], axis=0)
                     for nm in in_names]
        concat_zero = [np.zeros((n_cores * a.shape[0], *a.shape[1:]), a.dtype)
                       for a in out_avals]
        return [_jax.device_put(x, sharding) for x in concat_in + concat_zero]

    def call(staged):
        import jax as _jax
        outs = sharded(*staged)
        _jax.block_until_ready(outs)
        return outs

    def fetch(outs):
        return [{nm: np.asarray(outs[i]).reshape(n_cores, *out_avals[i].shape)[c]
                 for i, nm in enumerate(out_names)} for c in range(n_cores)]

    return stage, call, fetch


_CACHE = {}


def _get_compiled(st):
    key = (st["S_core"], st["C_CH"], tuple(st["cpg"].tolist()))
    if key not in _CACHE:
        nc = _build(st)
        _CACHE[key] = (nc, _make_runner(nc, NCORES))
    return _CACHE[key]


def _in_maps(st, per_core, x, inputs):
    node_slot = st["node_slot"]
    S_core = st["S_core"]
    hom_W = np.asarray(inputs["hom_W"], np.float32)
    het_W = np.asarray(inputs["het_W"], np.float32)
    hom_b = np.asarray(inputs["hom_b"], np.float32)
    het_b = np.asarray(inputs["het_b"], np.float32)
    w2 = np.ascontiguousarray(np.concatenate([hom_W[:, 2], het_W[:, 2]], axis=2))
    w1 = np.ascontiguousarray(np.concatenate([hom_W[:, 1], het_W[:, 1]], axis=2))
    w0 = np.ascontiguousarray(np.concatenate([hom_W[:, 0], het_W[:, 0]], axis=2))
    bb = np.ascontiguousarray(np.concatenate([hom_b, het_b], axis=1)[:, None, :])
    rcount = np.zeros((NGP, 1), np.float32)
    rcount[:N_GRAPHS, 0] = 1.0 / np.maximum(st["counts"], 1.0)

    x = np.asarray(x, np.float32)
    maps = []
    for c in range(NCORES):
        xo = np.zeros((S_core, F_IN), np.float32)
        m = (node_slot >= c * S_core) & (node_slot < (c + 1) * S_core)
        xo[node_slot[m] - c * S_core] = x[m]
        pc = per_core[c]
        maps.append({
            "x_own": xo, "srcA": pc["src"], "dlA": pc["dl"],
            "mhA": pc["mh"], "mtA": pc["mt"], "batchloc": pc["batchloc"],
            "maskrow": pc["maskrow"], "gidx": pc["gidx"], "rcount_g": rcount,
            "w2cat": w2, "w1cat": w1, "w0cat": w0, "bcat": bb,
            "lin1w": np.asarray(inputs["lin1_W"], np.float32),
            "lin1b": np.asarray(inputs["lin1_b"], np.float32)[:, None],
            "lin2w": np.asarray(inputs["lin2_W"], np.float32),
            "lin2b": np.asarray(inputs["lin2_b"], np.float32)[:, None],
            "lin3w": np.asarray(inputs["lin3_W"], np.float32),
            "lin3b": np.asarray(inputs["lin3_b"], np.float32)[:, None]})
    return maps


def kernel(**inputs):
    x = np.asarray(inputs["x"])
    edge_index = np.asarray(inputs["edge_index"])
    batch = np.asarray(inputs["batch"])
    st, per_core = _prep(edge_index, batch, inputs["hom_mask"], inputs["het_mask"])
    nc, (stage, call, fetch) = _get_compiled(st)
    maps = _in_maps(st, per_core, x, inputs)
    staged = stage(maps)
    outs = call(staged)
    return fetch(outs)[0]["out"].astype(np.float32)

